# revision 22
# baseline (speedup 1.0000x reference)
"""CoAttention kernel for Trainium2 (8 NeuronCores, data-parallel over batch).

Math (per sample): ta = relu(seq_a @ W + b), tb likewise.  The reference
mean-pools the [N, rv_len, M] affinity before softmax, and mean-pooling
commutes with the dot product:

    atob_scores[n, l] = mean_m( ta[n,l,:] . tb_all_tokens[m,:] )
                      = ta[n,l,:] . mean_m( tb_all_tokens[m,:] )

so each side only needs a dot with the *other side's per-sample mean
feature vector* — the 52M-element affinity tensor is never materialized.

v3 schedule.  Trace findings driving it: the DMA path is descriptor-rate
bound (~250-650ns per partition-row record per SDMA engine), so
per-partition rows must be fat and descriptor counts minimal; and the PE
queue is in-order, so a tail matmul whose input DMA is stuck behind bulk
traffic stalls all later FC matmuls.

- seq ships as fp16 (end-to-end rel-err ~4e-3 vs 2e-2 tolerance).
- sq01 (c0|c1) packs TWO samples per tile -> [128, 4*TPS] with 10KB
  rows; side b on sync, side a on gpsimd; 256 records per queue.
- c2 remainders ship per side as one [45, BPC*TPS] fat-row DMA on the
  scalar queue, where row 44 is all-ones: the FC bias is folded into
  the contraction (ta = [x|1] @ [W;b]), killing the [128,1]
  128-descriptor bias DMA.
- W ships as one [128, 3*DH] packed tile (128 records instead of 300);
  no identity matrix: the final transpose is a DVE 32x32 block
  transpose + block-permuted output DMAs.
- mask ships as an ADDITIVE f32 mask (0 / -1e9), so masking is one DVE
  add instead of memset+copy_predicated.
- scalar queue carries only: wpack, maskadd, sq2 x2, then per-sample
  tail smalls (scores reshape, out_w, weight rows) — a tail DMA never
  waits behind bulk.
- ~12 junk matmuls at t=0 warm the PE HAM clock gate (else the first
  ~3.4us of FC runs at 1.2GHz).
- weight broadcast is a PE ones-matmul into bank-sized PSUM chunks;
  DVE multiplies taT(fp16) into an fp16 tmp and does per-chunk
  segmented reduces into aoutT.
- PSUM: fc tag [128,1280] bufs=2 (6 banks, score matvec chunks ride
  the same ring), wbc tag [128,512] bufs=2 (2 banks) = exactly 8.
"""
import sys

sys.path.insert(0, "/opt/trn_rl_repo")

import numpy as np

import concourse.bacc as bacc
import concourse.tile as tile
from concourse import mybir

# Problem shape (hardcoded per contest contract)
BZ, RV, RL, DIN, DH = 32, 10, 128, 300, 128
NCORES = 8
BPC = BZ // NCORES            # samples per core: 4
TPS = RV * RL                 # tokens per sample: 1280
RPC = BPC * RV                # reviews per core: 40
NEG_INF = -1e9
KC2 = DIN - 2 * DH + 1        # c2 contraction rows incl. the ones row: 45

f32 = mybir.dt.float32
f32r = mybir.dt.float32r
f16 = mybir.dt.float16
AF = mybir.ActivationFunctionType
AX = mybir.AxisListType
ALU = mybir.AluOpType

# free-dim chunks of one sample's tokens (N <= 512 for one PSUM bank;
# chunk boundaries are review-aligned so segmented reduces stay clean)
NCH = [(0, 512), (512, 512), (1024, 256)]

_CACHE = {}


def _build(iters=1, serial=False, loop_n=0, stage=3):
    nc = bacc.Bacc("TRN2", target_bir_lowering=False, debug=False)

    sq01_d = {s: nc.dram_tensor(f"sq01_{s}", [2 * DH, 4 * TPS], f16,
                                kind="ExternalInput")
              for s in "ab"}
    sq2_d = {s: nc.dram_tensor(f"sq2_{s}", [KC2, BPC * TPS], f16,
                               kind="ExternalInput")
             for s in "ab"}
    # additive mask, pair-batched layout: partition (side, smp%2, r),
    # free (pair, l) — the whole softmax for one PAIR of samples is a
    # single [4RV, RL] slice at partition 0
    mska_d = nc.dram_tensor("maskadd", [4 * RV, 2 * RL], f32,
                            kind="ExternalInput")
    w_d = nc.dram_tensor("wpack", [DH, 3 * DH], f16, kind="ExternalInput")

    out_v = {s: nc.dram_tensor(f"out_{s}", [RPC, DH], f32, kind="ExternalOutput")
             for s in "ab"}
    out_w = {s: nc.dram_tensor(f"outw_{s}", [RPC, RL], f32, kind="ExternalOutput")
             for s in "ab"}

    import contextlib
    outer_tc = tile.TileContext(nc) if not serial else None
    with (outer_tc if outer_tc is not None else contextlib.nullcontext()):
      for it_ in range(iters):
        pfx = f"i{it_}_" if iters > 1 else ""
        with (
            tile.TileContext(nc) if serial else contextlib.nullcontext()
        ) as maybe_tc:
          tc = maybe_tc if serial else outer_tc
          with (
            tc.For_i(0, loop_n, 1) if loop_n else contextlib.nullcontext()
          ):
           with (
            tc.tile_pool(name=pfx + "cst", bufs=1) as cst,
            tc.tile_pool(name=pfx + "seq", bufs=1) as seqp,
            tc.tile_pool(name=pfx + "ta", bufs=8) as tap,
            tc.tile_pool(name=pfx + "sm", bufs=1) as smp_pool,
            tc.tile_pool(name=pfx + "ps", bufs=2, space="PSUM") as ps,
        ):
            # ---- scalar (HWDGE) queue: lean consts, then the two c2
            # fat-row batches; per-sample tail smalls follow later.
            w_t = cst.tile([DH, 3 * DH], f16, tag="w", name=pfx + "wpack")
            nc.scalar.dma_start(w_t[:], w_d[:])
            mskf = cst.tile([4 * RV, 2 * RL], f32, tag="mska",
                            name=pfx + "mskf")
            nc.scalar.dma_start(mskf[:], mska_d[:])

            def w_lhs(c):
                kw = DH if c < 2 else KC2
                return w_t[0:kw, c * DH:(c + 1) * DH]

            # ---- bulk seq stream: two-sample pair tiles, 10KB rows;
            # side b on sync, side a on gpsimd.  Each side's c2 batch
            # rides its own bulk queue BETWEEN pair0 and pair1 so FC(0)
            # unblocks early and the scalar queue stays small-only.
            sq01, sq2 = {}, {}
            qeng = {"b": nc.sync, "a": nc.gpsimd}
            for s in "ab":
                t01 = seqp.tile([DH, 4 * TPS], f16, tag="seq01",
                                bufs=4, name=f"{pfx}sq01_{s}0")
                qeng[s].dma_start(t01[:], sq01_d[s][0:DH, :])
                sq01[(s, 0)] = t01
            for s in "ab":
                sq2[s] = seqp.tile([KC2, BPC * TPS], f16, tag="seq2",
                                   bufs=2, name=f"{pfx}sq2_{s}")
                qeng[s].dma_start(sq2[s][:], sq2_d[s][:])
            for s in "ab":
                t01 = seqp.tile([DH, 4 * TPS], f16, tag="seq01",
                                bufs=4, name=f"{pfx}sq01_{s}1")
                qeng[s].dma_start(t01[:], sq01_d[s][DH:2 * DH, :])
                sq01[(s, 1)] = t01

            def sq_rhs(s, smp, c, n0, nw):
                if c < 2:
                    base = (smp % 2) * 2 * TPS + c * TPS
                    return sq01[(s, smp // 2)][:, base + n0:base + n0 + nw]
                return sq2[s][:, smp * TPS + n0:smp * TPS + n0 + nw]

            taT, acc, mean16, aoutT = {}, {}, {}, {}
            for s in "ab":
                acc[s] = cst.tile([DH, BPC], f32, tag=f"acc{s}", name=f"{pfx}acc_{s}")
                mean16[s] = cst.tile([DH, BPC], f16, tag=f"mean{s}",
                                     name=f"{pfx}mean_{s}")
                aoutT[s] = cst.tile([DH, RPC], f32, tag=f"aoutT{s}",
                                    name=f"{pfx}aoutT_{s}")
            w2d40 = cst.tile([4 * RV, 2 * RL], f32, tag="w2d",
                             name=pfx + "w2d40")
            ones1 = cst.tile([1, DH], f32, tag="ones", name=pfx + "ones1")
            nc.vector.memset(ones1[:], 1.0)
            # identity for the epilogue PE transpose, built on-chip (no
            # 128-descriptor const DMA): ones everywhere, then keep only
            # the j == p diagonal via an affine iota select
            ones_sq = cst.tile([DH, DH], f32, tag="onsq", name=pfx + "ones_sq")
            nc.vector.memset(ones_sq[:], 1.0)
            ident_t = cst.tile([DH, DH], f32, tag="ident", name=pfx + "ident_t")
            nc.gpsimd.affine_select(
                ident_t[:], ones_sq[:], pattern=[[1, DH]],
                compare_op=ALU.is_equal, fill=0.0,
                base=0, channel_multiplier=-1)

            # ---- PE warm-up: ~5us of junk matmuls so HAM un-throttles
            # the clock gate before the real FC arrives (data lands ~8us
            # in).  No input DMA dependency (junk lhsT from a memset);
            # writes a pool slot the first FC start=True resets.
            wrhs = smp_pool.tile([DH, 512], f16, tag="wrhs", name=pfx + "wrhs")
            nc.vector.memset(wrhs[:], 0.0)
            if stage >= 1:
                wfc = ps.tile([DH, TPS], f32, tag="fc", bufs=2,
                              name=pfx + "warm_fc")
                for k in range(12):
                    nc.tensor.matmul(wfc[:, 0:512], wrhs[:, 0:DH], wrhs[:],
                                     start=True, stop=True)

            other = {"a": "b", "b": "a"}

            def emit_fc_pair(smp):
                if stage < 1:
                    return
                pfc = {}
                for s in ("b", "a"):
                    pfc[s] = ps.tile([DH, TPS], f32, tag="fc", bufs=2,
                                     name=f"{pfx}pfc_{s}{smp}")
                    taT[(s, smp)] = tap.tile([DH, TPS], f16, tag="taT",
                                             name=f"{pfx}taT_{s}{smp}")
                # c-outer: 3 weight loads per sample pair instead of 18
                for c in range(3):
                    for s in ("b", "a"):
                        for n0, nw in NCH:
                            nc.tensor.matmul(
                                pfc[s][:, n0:n0 + nw],
                                w_lhs(c),
                                sq_rhs(s, smp, c, n0, nw),
                                start=(c == 0), stop=(c == 2))
                for s in ("b", "a"):
                    nc.scalar.activation(
                        taT[(s, smp)][:], pfc[s][:], AF.Relu,
                        accum_out=acc[s][:, smp:smp + 1])
                    nc.scalar.mul(mean16[s][:, smp:smp + 1],
                                  acc[s][:, smp:smp + 1], 1.0 / TPS)

            srow_p = {}

            def emit_scores(smp):
                if stage < 2:
                    return
                # scores: M=1 fp16 matvec against the other side's mean,
                # in bank-sized PSUM chunks riding the fc tag ring; the
                # score row lands in the PAIR's [1, 2*2*TPS] row at
                # (side, smp%2)-major offsets
                p, j = smp // 2, smp % 2
                if j == 0:
                    srow_p[p] = smp_pool.tile([1, 4 * TPS], f32, tag="srow",
                                              bufs=2, name=f"{pfx}srow_p{p}")
                srow = srow_p[p]
                for i, s in enumerate(("a", "b")):
                    off = (i * 2 + j) * TPS
                    for ci, (n0, nw) in enumerate(NCH):
                        pscc = ps.tile([1, 512], f32, tag="fc", bufs=2,
                                       name=f"{pfx}psc_{s}{smp}{ci}")
                        nc.tensor.matmul(
                            pscc[:, :nw],
                            mean16[other[s]][:, smp:smp + 1],
                            taT[(s, smp)][:, n0:n0 + nw])
                        nc.scalar.copy(srow[:, off + n0:off + n0 + nw],
                                       pscc[:, :nw])

            def emit_soft_pair(p):
                # ONE batched masked softmax for both samples of the
                # pair: [4RV, RL] with partition = (side, smp%2, review).
                # Single chain of cross-engine hops per PAIR instead of
                # per sample.
                scs = smp_pool.tile([4 * RV, RL], f32, tag="scs", bufs=2,
                                    name=f"{pfx}scs_p{p}")
                nc.scalar.dma_start(scs[:], srow_p[p][:])
                lgs = smp_pool.tile([4 * RV, RL], f32, tag="lgs", bufs=2,
                                    name=f"{pfx}lgs_p{p}")
                nc.vector.tensor_tensor(
                    out=lgs[:], in0=scs[:],
                    in1=mskf[:, p * RL:(p + 1) * RL], op=ALU.add)
                negmax = smp_pool.tile([4 * RV, 1], f32, tag="negmax", bufs=2,
                                       name=f"{pfx}negmax_p{p}")
                nc.vector.reduce_max(out=negmax[:], in_=lgs[:],
                                     axis=AX.X, negate=True)
                e2d = smp_pool.tile([4 * RV, RL], f32, tag="e2d", bufs=2,
                                    name=f"{pfx}e2d_p{p}")
                ssum = smp_pool.tile([4 * RV, 1], f32, tag="ssum", bufs=2,
                                     name=f"{pfx}ssum_p{p}")
                nc.scalar.activation(e2d[:], lgs[:], AF.Exp, bias=negmax[:],
                                     accum_out=ssum[:])
                rec = smp_pool.tile([4 * RV, 1], f32, tag="rec", bufs=2,
                                    name=f"{pfx}rec_p{p}")
                nc.vector.reciprocal(rec[:], ssum[:])
                nc.vector.tensor_scalar_mul(
                    w2d40[:, p * RL:(p + 1) * RL], e2d[:], rec[:])
                # ship softmax weights (2 DMAs/pair; rows are already in
                # (smp, review)-major order per side) + the 4 flattened
                # weight rows for the weighted sums
                wrows = {}
                for i, s in enumerate(("a", "b")):
                    nc.scalar.dma_start(
                        out_w[s][p * 2 * RV:(p + 1) * 2 * RV, :],
                        w2d40[i * 2 * RV:(i + 1) * 2 * RV,
                              p * RL:(p + 1) * RL])
                    for j in range(2):
                        wr = smp_pool.tile([1, TPS], f32, tag=f"wrow{s}",
                                           bufs=2,
                                           name=f"{pfx}wrow_{s}{2 * p + j}")
                        nc.scalar.dma_start(
                            wr[:], w2d40[(i * 2 + j) * RV:
                                         (i * 2 + j + 1) * RV,
                                         p * RL:(p + 1) * RL])
                        wrows[(s, 2 * p + j)] = wr
                return wrows

            def emit_wsum(smp, wrow):
                if stage < 3:
                    return
                # weighted sums: PE ones-matmul broadcasts the weight row
                # into bank-sized PSUM chunks; DVE multiplies with
                # taT(fp16) into fp16 tmp; per-chunk segmented reduces
                # (chunks are review-aligned: 4+4+2 reviews)
                for i, s in enumerate(("a", "b")):
                    tmp = smp_pool.tile([DH, TPS], f16, tag="tmp", bufs=2,
                                        name=f"{pfx}tmp_{s}{smp}")
                    for ci, (n0, nw) in enumerate(NCH):
                        wbc = ps.tile([DH, 512], f32, tag="wbc", bufs=2,
                                      name=f"{pfx}wbc_{s}{smp}{ci}")
                        nc.tensor.matmul(
                            wbc[:, :nw],
                            ones1[:].bitcast(f32r),
                            wrow[s][:, n0:n0 + nw].bitcast(f32r))
                        nc.vector.tensor_tensor(
                            out=tmp[:, n0:n0 + nw],
                            in0=taT[(s, smp)][:, n0:n0 + nw],
                            in1=wbc[:, :nw], op=ALU.mult)
                        nc.vector.reduce_sum(
                            out=aoutT[s][:, smp * RV + n0 // RL:
                                         smp * RV + (n0 + nw) // RL],
                            in_=tmp[:, n0:n0 + nw].rearrange(
                                "p (r l) -> p r l", l=RL),
                            axis=AX.X)

            # Emission order: per pair p, FC+scores for its two samples,
            # then the pair's single batched softmax chain, then the
            # pair's weighted sums — emitted BEFORE the next pair's FC
            # so the in-order PE queue never back-waits on a tail DMA
            # that is later than its own pair's data.
            for p in range(BPC // 2):
                for j in range(2):
                    emit_fc_pair(2 * p + j)
                    emit_scores(2 * p + j)
                if stage >= 2:
                    wrows = emit_soft_pair(p)
                    for j in range(2):
                        emit_wsum(2 * p + j,
                                  {s: wrows[(s, 2 * p + j)] for s in "ab"})

            # ---- per-side epilogue: PE transpose (on-chip identity),
            # DVE copy out of PSUM, out_v on the idle sync queue
            for s in ("a", "b") if stage >= 3 else ():
                ptp = ps.tile([RPC, DH], f32, tag="wbc", bufs=2,
                              name=f"{pfx}ptp_{s}")
                nc.tensor.matmul(ptp[:], aoutT[s][:], ident_t[:],
                                 is_transpose=True)
                aout = smp_pool.tile([RPC, DH], f32, tag="aout",
                                     name=f"{pfx}aout_{s}")
                nc.vector.tensor_copy(aout[:], ptp[:])
                nc.sync.dma_start(out_v[s][:], aout[:])

    nc.compile()
    return nc


def build_in_maps(seq_a, seq_b, mask_a, mask_b, W, b):
    seq_a = np.asarray(seq_a, dtype=np.float32)
    seq_b = np.asarray(seq_b, dtype=np.float32)
    mask_a = np.asarray(mask_a, dtype=np.int32)
    mask_b = np.asarray(mask_b, dtype=np.int32)
    W = np.asarray(W, dtype=np.float32)
    b = np.asarray(b, dtype=np.float32)

    # W packed [128, 3*DH] fp16 with the bias folded in as c2 row 44
    wpack = np.zeros((DH, 3 * DH), dtype=np.float16)
    wpack[:, 0:DH] = W[0:DH]
    wpack[:, DH:2 * DH] = W[DH:2 * DH]
    wpack[0:DIN - 2 * DH, 2 * DH:3 * DH] = W[2 * DH:DIN]
    wpack[KC2 - 1, 2 * DH:3 * DH] = b

    in_maps = []
    for core in range(NCORES):
        b0 = core * BPC
        sl = {}
        for name, seq in (("a", seq_a), ("b", seq_b)):
            # [BPC, TPS, DIN] -> [BPC, DIN, TPS] fp16; c0|c1 of two
            # samples concatenated column-wise into [2*128, 4*TPS]; c2
            # batched across the 4 samples into [45, BPC*TPS] with a
            # trailing all-ones row (bias fold)
            chunk = (seq[b0:b0 + BPC].reshape(BPC, TPS, DIN)
                     .transpose(0, 2, 1).astype(np.float16))
            c01 = np.concatenate(
                [chunk[:, 0:DH, :], chunk[:, DH:2 * DH, :]], axis=2)
            sl[f"sq01_{name}"] = np.ascontiguousarray(np.concatenate(
                [np.concatenate([c01[2 * p], c01[2 * p + 1]], axis=1)
                 for p in range(2)], axis=0))
            c2 = chunk[:, 2 * DH:DIN, :].transpose(1, 0, 2).reshape(
                DIN - 2 * DH, BPC * TPS)
            sl[f"sq2_{name}"] = np.ascontiguousarray(np.concatenate(
                [c2, np.ones((1, BPC * TPS), dtype=np.float16)], axis=0))
        # [4RV, 2RL]: partition (side, smp%2, review), free (pair, token)
        msk = np.stack([m[b0:b0 + BPC].reshape(2, 2, RV, RL)
                        for m in (mask_a, mask_b)])  # [side, pair, j, r, l]
        msk = np.ascontiguousarray(
            msk.transpose(0, 2, 3, 1, 4).reshape(4 * RV, 2 * RL))
        sl["maskadd"] = np.where(msk > 0, 0.0, NEG_INF).astype(np.float32)
        sl["wpack"] = wpack
        in_maps.append(sl)
    return in_maps


def kernel(seq_a, seq_b, mask_a, mask_b, W, b):
    if "nc" not in _CACHE:
        _CACHE["nc"] = _build()
    nc = _CACHE["nc"]
    in_maps = build_in_maps(seq_a, seq_b, mask_a, mask_b, W, b)

    from concourse.bass_utils import run_bass_kernel_spmd
    res = run_bass_kernel_spmd(nc, in_maps, core_ids=list(range(NCORES)))
    _CACHE["last_result"] = res

    a_out = np.concatenate([r["out_a"] for r in res.results], axis=0)
    b_out = np.concatenate([r["out_b"] for r in res.results], axis=0)
    atob_w = np.concatenate([r["outw_a"] for r in res.results], axis=0)
    btoa_w = np.concatenate([r["outw_b"] for r in res.results], axis=0)
    return (a_out, b_out, atob_w, btoa_w)


# revision 27
# speedup vs baseline: 1.0933x; 1.0933x over previous
"""CoAttention kernel for Trainium2 (8 NeuronCores, data-parallel over batch).

Math (per sample): ta = relu(seq_a @ W + b), tb likewise.  The reference
mean-pools the [N, rv_len, M] affinity before softmax, and mean-pooling
commutes with the dot product:

    atob_scores[n, l] = mean_m( ta[n,l,:] . tb_all_tokens[m,:] )
                      = ta[n,l,:] . mean_m( tb_all_tokens[m,:] )

so each side only needs a dot with the *other side's per-sample mean
feature vector* — the 52M-element affinity tensor is never materialized.

v3 schedule.  Trace findings driving it: the DMA path is descriptor-rate
bound (~250-650ns per partition-row record per SDMA engine), so
per-partition rows must be fat and descriptor counts minimal; and the PE
queue is in-order, so a tail matmul whose input DMA is stuck behind bulk
traffic stalls all later FC matmuls.

- seq ships as fp16 (end-to-end rel-err ~4e-3 vs 2e-2 tolerance).
- sq01 (c0|c1) packs TWO samples per tile -> [128, 4*TPS] with 10KB
  rows; side b on sync, side a on gpsimd; 256 records per queue.
- c2 remainders ship per side as one [45, BPC*TPS] fat-row DMA on the
  scalar queue, where row 44 is all-ones: the FC bias is folded into
  the contraction (ta = [x|1] @ [W;b]), killing the [128,1]
  128-descriptor bias DMA.
- W ships as one [128, 3*DH] packed tile (128 records instead of 300);
  no identity matrix: the final transpose is a DVE 32x32 block
  transpose + block-permuted output DMAs.
- mask ships as an ADDITIVE f32 mask (0 / -1e9), so masking is one DVE
  add instead of memset+copy_predicated.
- scalar queue carries only: wpack, maskadd, sq2 x2, then per-sample
  tail smalls (scores reshape, out_w, weight rows) — a tail DMA never
  waits behind bulk.
- ~12 junk matmuls at t=0 warm the PE HAM clock gate (else the first
  ~3.4us of FC runs at 1.2GHz).
- weight broadcast is a PE ones-matmul into bank-sized PSUM chunks;
  DVE multiplies taT(fp16) into an fp16 tmp and does per-chunk
  segmented reduces into aoutT.
- PSUM: fc tag [128,1280] bufs=2 (6 banks, score matvec chunks ride
  the same ring), wbc tag [128,512] bufs=2 (2 banks) = exactly 8.
"""
import sys

sys.path.insert(0, "/opt/trn_rl_repo")

import numpy as np

import concourse.bacc as bacc
import concourse.tile as tile
from concourse import mybir

# Problem shape (hardcoded per contest contract)
BZ, RV, RL, DIN, DH = 32, 10, 128, 300, 128
NCORES = 8
BPC = BZ // NCORES            # samples per core: 4
TPS = RV * RL                 # tokens per sample: 1280
RPC = BPC * RV                # reviews per core: 40
NEG_INF = -1e9
KC2 = DIN - 2 * DH + 1        # c2 contraction rows incl. the ones row: 45

f32 = mybir.dt.float32
f32r = mybir.dt.float32r
f16 = mybir.dt.float16
AF = mybir.ActivationFunctionType
AX = mybir.AxisListType
ALU = mybir.AluOpType

# free-dim chunks of one sample's tokens (N <= 512 for one PSUM bank;
# chunk boundaries are review-aligned so segmented reduces stay clean)
NCH = [(0, 512), (512, 512), (1024, 256)]

_CACHE = {}


def _build(iters=1, serial=False, loop_n=0, stage=3):
    nc = bacc.Bacc("TRN2", target_bir_lowering=False, debug=False)

    sq01_d = {s: nc.dram_tensor(f"sq01_{s}", [BPC * DH, 2 * TPS], f16,
                                kind="ExternalInput")
              for s in "ab"}
    sq2_d = {s: nc.dram_tensor(f"sq2_{s}", [KC2, BPC * TPS], f16,
                               kind="ExternalInput")
             for s in "ab"}
    # additive mask, pair-batched layout: partition (side, smp%2, r),
    # free (pair, l) — the whole softmax for one PAIR of samples is a
    # single [4RV, RL] slice at partition 0
    mska_d = nc.dram_tensor("maskadd", [4 * RV, 2 * RL], f32,
                            kind="ExternalInput")
    w_d = nc.dram_tensor("wpack", [DH, 3 * DH], f16, kind="ExternalInput")

    out_v = {s: nc.dram_tensor(f"out_{s}", [RPC, DH], f32, kind="ExternalOutput")
             for s in "ab"}
    out_w = {s: nc.dram_tensor(f"outw_{s}", [RPC, RL], f32, kind="ExternalOutput")
             for s in "ab"}

    import contextlib
    outer_tc = tile.TileContext(nc) if not serial else None
    with (outer_tc if outer_tc is not None else contextlib.nullcontext()):
      for it_ in range(iters):
        pfx = f"i{it_}_" if iters > 1 else ""
        with (
            tile.TileContext(nc) if serial else contextlib.nullcontext()
        ) as maybe_tc:
          tc = maybe_tc if serial else outer_tc
          with (
            tc.For_i(0, loop_n, 1) if loop_n else contextlib.nullcontext()
          ):
           with (
            tc.tile_pool(name=pfx + "cst", bufs=1) as cst,
            tc.tile_pool(name=pfx + "seq", bufs=1) as seqp,
            tc.tile_pool(name=pfx + "ta", bufs=8) as tap,
            tc.tile_pool(name=pfx + "sm", bufs=1) as smp_pool,
            tc.tile_pool(name=pfx + "ps", bufs=2, space="PSUM") as ps,
        ):
            # ---- scalar (HWDGE) queue: lean consts, then the two c2
            # fat-row batches; per-sample tail smalls follow later.
            w_t = cst.tile([DH, 3 * DH], f16, tag="w", name=pfx + "wpack")
            nc.scalar.dma_start(w_t[:], w_d[:])
            mskf = cst.tile([4 * RV, 2 * RL], f32, tag="mska",
                            name=pfx + "mskf")
            nc.scalar.dma_start(mskf[:], mska_d[:])

            def w_lhs(c):
                kw = DH if c < 2 else KC2
                return w_t[0:kw, c * DH:(c + 1) * DH]

            # ---- bulk seq stream: per-sample [128, 2*TPS] tiles, side
            # b on sync, side a on gpsimd.  Each side's c2 batch rides
            # its own bulk queue right after sample 0 so FC(0) unblocks
            # early and the scalar queue stays small-only.
            sq01, sq2 = {}, {}
            qeng = {"b": nc.sync, "a": nc.gpsimd}

            def emit_sq01(s, smp):
                t01 = seqp.tile([DH, 2 * TPS], f16, tag="seq01",
                                bufs=4, name=f"{pfx}sq01_{s}{smp}")
                qeng[s].dma_start(
                    t01[:], sq01_d[s][smp * DH:(smp + 1) * DH, :])
                sq01[(s, smp)] = t01

            for s in "ab":
                emit_sq01(s, 0)
            for s in "ab":
                sq2[s] = seqp.tile([KC2, BPC * TPS], f16, tag="seq2",
                                   bufs=2, name=f"{pfx}sq2_{s}")
                qeng[s].dma_start(sq2[s][:], sq2_d[s][:])
            for smp in range(1, BPC):
                for s in "ab":
                    emit_sq01(s, smp)

            def sq_rhs(s, smp, c, n0, nw):
                if c < 2:
                    return sq01[(s, smp)][:, c * TPS + n0:c * TPS + n0 + nw]
                return sq2[s][:, smp * TPS + n0:smp * TPS + n0 + nw]

            taT, acc, mean16, aoutT = {}, {}, {}, {}
            for s in "ab":
                acc[s] = cst.tile([DH, BPC], f32, tag=f"acc{s}", name=f"{pfx}acc_{s}")
                mean16[s] = cst.tile([DH, BPC], f16, tag=f"mean{s}",
                                     name=f"{pfx}mean_{s}")
                aoutT[s] = cst.tile([DH, RPC], f32, tag=f"aoutT{s}",
                                    name=f"{pfx}aoutT_{s}")
            w2d40 = cst.tile([4 * RV, 2 * RL], f32, tag="w2d",
                             name=pfx + "w2d40")
            ones1 = cst.tile([1, DH], f32, tag="ones", name=pfx + "ones1")
            nc.vector.memset(ones1[:], 1.0)
            # identity for the epilogue PE transpose, built on-chip (no
            # 128-descriptor const DMA): ones everywhere, then keep only
            # the j == p diagonal via an affine iota select
            ones_sq = cst.tile([DH, DH], f32, tag="onsq", name=pfx + "ones_sq")
            nc.vector.memset(ones_sq[:], 1.0)
            ident_t = cst.tile([DH, DH], f32, tag="ident", name=pfx + "ident_t")
            nc.gpsimd.affine_select(
                ident_t[:], ones_sq[:], pattern=[[1, DH]],
                compare_op=ALU.is_equal, fill=0.0,
                base=0, channel_multiplier=-1)

            # ---- PE warm-up: ~5us of junk matmuls so HAM un-throttles
            # the clock gate before the real FC arrives (data lands ~8us
            # in).  No input DMA dependency (junk lhsT from a memset);
            # writes a pool slot the first FC start=True resets.
            wrhs = smp_pool.tile([DH, 512], f16, tag="wrhs", name=pfx + "wrhs")
            nc.vector.memset(wrhs[:], 0.0)
            if stage >= 1:
                wfc = ps.tile([DH, TPS], f32, tag="fc", bufs=2,
                              name=pfx + "warm_fc")
                for k in range(12):
                    nc.tensor.matmul(wfc[:, 0:512], wrhs[:, 0:DH], wrhs[:],
                                     start=True, stop=True)

            other = {"a": "b", "b": "a"}

            def emit_fc_pair(smp):
                if stage < 1:
                    return
                pfc = {}
                for s in ("b", "a"):
                    pfc[s] = ps.tile([DH, TPS], f32, tag="fc", bufs=2,
                                     name=f"{pfx}pfc_{s}{smp}")
                    taT[(s, smp)] = tap.tile([DH, TPS], f16, tag="taT",
                                             name=f"{pfx}taT_{s}{smp}")
                # c-outer: 3 weight loads per sample pair instead of 18
                for c in range(3):
                    for s in ("b", "a"):
                        for n0, nw in NCH:
                            nc.tensor.matmul(
                                pfc[s][:, n0:n0 + nw],
                                w_lhs(c),
                                sq_rhs(s, smp, c, n0, nw),
                                start=(c == 0), stop=(c == 2))
                for s in ("b", "a"):
                    nc.scalar.activation(
                        taT[(s, smp)][:], pfc[s][:], AF.Relu,
                        accum_out=acc[s][:, smp:smp + 1])
                    nc.scalar.mul(mean16[s][:, smp:smp + 1],
                                  acc[s][:, smp:smp + 1], 1.0 / TPS)

            srow_p = {}

            def emit_scores(smp):
                if stage < 2:
                    return
                # scores: M=1 fp16 matvec against the other side's mean,
                # in bank-sized PSUM chunks riding the fc tag ring; the
                # score row lands in the PAIR's [1, 2*2*TPS] row at
                # (side, smp%2)-major offsets
                p, j = smp // 2, smp % 2
                if j == 0:
                    srow_p[p] = smp_pool.tile([1, 4 * TPS], f32, tag="srow",
                                              bufs=2, name=f"{pfx}srow_p{p}")
                srow = srow_p[p]
                for i, s in enumerate(("a", "b")):
                    off = (i * 2 + j) * TPS
                    for ci, (n0, nw) in enumerate(NCH):
                        pscc = ps.tile([1, 512], f32, tag="wbc", bufs=2,
                                       name=f"{pfx}psc_{s}{smp}{ci}")
                        nc.tensor.matmul(
                            pscc[:, :nw],
                            mean16[other[s]][:, smp:smp + 1],
                            taT[(s, smp)][:, n0:n0 + nw])
                        nc.scalar.copy(srow[:, off + n0:off + n0 + nw],
                                       pscc[:, :nw])

            def emit_soft_pair(p):
                # ONE batched masked softmax for both samples of the
                # pair: [4RV, RL] with partition = (side, smp%2, review).
                # Single chain of cross-engine hops per PAIR instead of
                # per sample.
                scs = smp_pool.tile([4 * RV, RL], f32, tag="scs", bufs=2,
                                    name=f"{pfx}scs_p{p}")
                nc.scalar.dma_start(scs[:], srow_p[p][:])
                lgs = smp_pool.tile([4 * RV, RL], f32, tag="lgs", bufs=2,
                                    name=f"{pfx}lgs_p{p}")
                nc.vector.tensor_tensor(
                    out=lgs[:], in0=scs[:],
                    in1=mskf[:, p * RL:(p + 1) * RL], op=ALU.add)
                negmax = smp_pool.tile([4 * RV, 1], f32, tag="negmax", bufs=2,
                                       name=f"{pfx}negmax_p{p}")
                nc.vector.reduce_max(out=negmax[:], in_=lgs[:],
                                     axis=AX.X, negate=True)
                e2d = smp_pool.tile([4 * RV, RL], f32, tag="e2d", bufs=2,
                                    name=f"{pfx}e2d_p{p}")
                ssum = smp_pool.tile([4 * RV, 1], f32, tag="ssum", bufs=2,
                                     name=f"{pfx}ssum_p{p}")
                nc.scalar.activation(e2d[:], lgs[:], AF.Exp, bias=negmax[:],
                                     accum_out=ssum[:])
                rec = smp_pool.tile([4 * RV, 1], f32, tag="rec", bufs=2,
                                    name=f"{pfx}rec_p{p}")
                nc.vector.reciprocal(rec[:], ssum[:])
                nc.vector.tensor_scalar_mul(
                    w2d40[:, p * RL:(p + 1) * RL], e2d[:], rec[:])
                # ship softmax weights (2 DMAs/pair; rows are already in
                # (smp, review)-major order per side) + the 4 flattened
                # weight rows for the weighted sums
                wrows = {}
                for i, s in enumerate(("a", "b")):
                    nc.scalar.dma_start(
                        out_w[s][p * 2 * RV:(p + 1) * 2 * RV, :],
                        w2d40[i * 2 * RV:(i + 1) * 2 * RV,
                              p * RL:(p + 1) * RL])
                    for j in range(2):
                        wr = smp_pool.tile([1, TPS], f32, tag=f"wrow{s}",
                                           bufs=2,
                                           name=f"{pfx}wrow_{s}{2 * p + j}")
                        nc.scalar.dma_start(
                            wr[:], w2d40[(i * 2 + j) * RV:
                                         (i * 2 + j + 1) * RV,
                                         p * RL:(p + 1) * RL])
                        wrows[(s, 2 * p + j)] = wr
                return wrows

            def emit_wsum(smp, wrow):
                if stage < 3:
                    return
                # weighted sums: PE ones-matmul broadcasts the weight row
                # into bank-sized PSUM chunks; DVE multiplies with
                # taT(fp16) into fp16 tmp; per-chunk segmented reduces
                # (chunks are review-aligned: 4+4+2 reviews)
                for i, s in enumerate(("a", "b")):
                    tmp = smp_pool.tile([DH, TPS], f16, tag="tmp", bufs=2,
                                        name=f"{pfx}tmp_{s}{smp}")
                    for ci, (n0, nw) in enumerate(NCH):
                        wbc = ps.tile([DH, 512], f32, tag="wbc", bufs=2,
                                      name=f"{pfx}wbc_{s}{smp}{ci}")
                        nc.tensor.matmul(
                            wbc[:, :nw],
                            ones1[:].bitcast(f32r),
                            wrow[s][:, n0:n0 + nw].bitcast(f32r))
                        nc.vector.tensor_tensor(
                            out=tmp[:, n0:n0 + nw],
                            in0=taT[(s, smp)][:, n0:n0 + nw],
                            in1=wbc[:, :nw], op=ALU.mult)
                        nc.vector.reduce_sum(
                            out=aoutT[s][:, smp * RV + n0 // RL:
                                         smp * RV + (n0 + nw) // RL],
                            in_=tmp[:, n0:n0 + nw].rearrange(
                                "p (r l) -> p r l", l=RL),
                            axis=AX.X)

            # Emission order: FC+scores per sample; pair 0's batched
            # softmax right after SC(1); its weighted sums AFTER FC(2)
            # so the wbc matmuls (gated on pair-0's softmax chain) slot
            # between FC groups without risking a stall of later FC.
            if stage >= 2:
                emit_fc_pair(0)
                emit_scores(0)
                emit_fc_pair(1)
                emit_scores(1)
                wr0 = emit_soft_pair(0)
                emit_fc_pair(2)
                emit_scores(2)
                for j in range(2):
                    emit_wsum(j, {s: wr0[(s, j)] for s in "ab"})
                emit_fc_pair(3)
                emit_scores(3)
                wr1 = emit_soft_pair(1)
                for j in range(2, 4):
                    emit_wsum(j, {s: wr1[(s, j)] for s in "ab"})
            else:
                for smp in range(BPC):
                    emit_fc_pair(smp)

            # ---- per-side epilogue: PE transpose (on-chip identity),
            # DVE copy out of PSUM, out_v on the idle sync queue
            for s in ("a", "b") if stage >= 3 else ():
                ptp = ps.tile([RPC, DH], f32, tag="wbc", bufs=2,
                              name=f"{pfx}ptp_{s}")
                nc.tensor.matmul(ptp[:], aoutT[s][:], ident_t[:],
                                 is_transpose=True)
                aout = smp_pool.tile([RPC, DH], f32, tag="aout",
                                     name=f"{pfx}aout_{s}")
                nc.vector.tensor_copy(aout[:], ptp[:])
                nc.sync.dma_start(out_v[s][:], aout[:])

    nc.compile()
    return nc


def build_in_maps(seq_a, seq_b, mask_a, mask_b, W, b):
    seq_a = np.asarray(seq_a, dtype=np.float32)
    seq_b = np.asarray(seq_b, dtype=np.float32)
    mask_a = np.asarray(mask_a, dtype=np.int32)
    mask_b = np.asarray(mask_b, dtype=np.int32)
    W = np.asarray(W, dtype=np.float32)
    b = np.asarray(b, dtype=np.float32)

    # W packed [128, 3*DH] fp16 with the bias folded in as c2 row 44
    wpack = np.zeros((DH, 3 * DH), dtype=np.float16)
    wpack[:, 0:DH] = W[0:DH]
    wpack[:, DH:2 * DH] = W[DH:2 * DH]
    wpack[0:DIN - 2 * DH, 2 * DH:3 * DH] = W[2 * DH:DIN]
    wpack[KC2 - 1, 2 * DH:3 * DH] = b

    in_maps = []
    for core in range(NCORES):
        b0 = core * BPC
        sl = {}
        for name, seq in (("a", seq_a), ("b", seq_b)):
            # [BPC, TPS, DIN] -> [BPC, DIN, TPS] fp16; c0|c1 of two
            # samples concatenated column-wise into [2*128, 4*TPS]; c2
            # batched across the 4 samples into [45, BPC*TPS] with a
            # trailing all-ones row (bias fold)
            chunk = (seq[b0:b0 + BPC].reshape(BPC, TPS, DIN)
                     .transpose(0, 2, 1).astype(np.float16))
            c01 = np.concatenate(
                [chunk[:, 0:DH, :], chunk[:, DH:2 * DH, :]], axis=2)
            sl[f"sq01_{name}"] = np.ascontiguousarray(
                c01.reshape(BPC * DH, 2 * TPS))
            c2 = chunk[:, 2 * DH:DIN, :].transpose(1, 0, 2).reshape(
                DIN - 2 * DH, BPC * TPS)
            sl[f"sq2_{name}"] = np.ascontiguousarray(np.concatenate(
                [c2, np.ones((1, BPC * TPS), dtype=np.float16)], axis=0))
        # [4RV, 2RL]: partition (side, smp%2, review), free (pair, token)
        msk = np.stack([m[b0:b0 + BPC].reshape(2, 2, RV, RL)
                        for m in (mask_a, mask_b)])  # [side, pair, j, r, l]
        msk = np.ascontiguousarray(
            msk.transpose(0, 2, 3, 1, 4).reshape(4 * RV, 2 * RL))
        sl["maskadd"] = np.where(msk > 0, 0.0, NEG_INF).astype(np.float32)
        sl["wpack"] = wpack
        in_maps.append(sl)
    return in_maps


def kernel(seq_a, seq_b, mask_a, mask_b, W, b):
    if "nc" not in _CACHE:
        _CACHE["nc"] = _build()
    nc = _CACHE["nc"]
    in_maps = build_in_maps(seq_a, seq_b, mask_a, mask_b, W, b)

    from concourse.bass_utils import run_bass_kernel_spmd
    res = run_bass_kernel_spmd(nc, in_maps, core_ids=list(range(NCORES)))
    _CACHE["last_result"] = res

    a_out = np.concatenate([r["out_a"] for r in res.results], axis=0)
    b_out = np.concatenate([r["out_b"] for r in res.results], axis=0)
    atob_w = np.concatenate([r["outw_a"] for r in res.results], axis=0)
    btoa_w = np.concatenate([r["outw_b"] for r in res.results], axis=0)
    return (a_out, b_out, atob_w, btoa_w)


# revision 31
# speedup vs baseline: 1.1172x; 1.0219x over previous
"""CoAttention kernel for Trainium2 (8 NeuronCores, data-parallel over batch).

Math (per sample): ta = relu(seq_a @ W + b), tb likewise.  The reference
mean-pools the [N, rv_len, M] affinity before softmax, and mean-pooling
commutes with the dot product:

    atob_scores[n, l] = mean_m( ta[n,l,:] . tb_all_tokens[m,:] )
                      = ta[n,l,:] . mean_m( tb_all_tokens[m,:] )

so each side only needs a dot with the *other side's per-sample mean
feature vector* — the 52M-element affinity tensor is never materialized.

v3 schedule.  Trace findings driving it: the DMA path is descriptor-rate
bound (~250-650ns per partition-row record per SDMA engine), so
per-partition rows must be fat and descriptor counts minimal; and the PE
queue is in-order, so a tail matmul whose input DMA is stuck behind bulk
traffic stalls all later FC matmuls.

- seq ships as fp16 (end-to-end rel-err ~4e-3 vs 2e-2 tolerance).
- sq01 (c0|c1) packs TWO samples per tile -> [128, 4*TPS] with 10KB
  rows; side b on sync, side a on gpsimd; 256 records per queue.
- c2 remainders ship per side as one [45, BPC*TPS] fat-row DMA on the
  scalar queue, where row 44 is all-ones: the FC bias is folded into
  the contraction (ta = [x|1] @ [W;b]), killing the [128,1]
  128-descriptor bias DMA.
- W ships as one [128, 3*DH] packed tile (128 records instead of 300);
  no identity matrix: the final transpose is a DVE 32x32 block
  transpose + block-permuted output DMAs.
- mask ships as an ADDITIVE f32 mask (0 / -1e9), so masking is one DVE
  add instead of memset+copy_predicated.
- scalar queue carries only: wpack, maskadd, sq2 x2, then per-sample
  tail smalls (scores reshape, out_w, weight rows) — a tail DMA never
  waits behind bulk.
- ~12 junk matmuls at t=0 warm the PE HAM clock gate (else the first
  ~3.4us of FC runs at 1.2GHz).
- weight broadcast is a PE ones-matmul into bank-sized PSUM chunks;
  DVE multiplies taT(fp16) into an fp16 tmp and does per-chunk
  segmented reduces into aoutT.
- PSUM: fc tag [128,1280] bufs=2 (6 banks, score matvec chunks ride
  the same ring), wbc tag [128,512] bufs=2 (2 banks) = exactly 8.
"""
import sys

sys.path.insert(0, "/opt/trn_rl_repo")

import numpy as np

import concourse.bacc as bacc
import concourse.tile as tile
from concourse import mybir

# Problem shape (hardcoded per contest contract)
BZ, RV, RL, DIN, DH = 32, 10, 128, 300, 128
NCORES = 8
BPC = BZ // NCORES            # samples per core: 4
TPS = RV * RL                 # tokens per sample: 1280
RPC = BPC * RV                # reviews per core: 40
NEG_INF = -1e9
KC2 = DIN - 2 * DH + 1        # c2 contraction rows incl. the ones row: 45

f32 = mybir.dt.float32
f32r = mybir.dt.float32r
f16 = mybir.dt.float16
AF = mybir.ActivationFunctionType
AX = mybir.AxisListType
ALU = mybir.AluOpType

# free-dim chunks of one sample's tokens (N <= 512 for one PSUM bank;
# chunk boundaries are review-aligned so segmented reduces stay clean)
NCH = [(0, 512), (512, 512), (1024, 256)]

_CACHE = {}


def _build(iters=1, serial=False, loop_n=0, stage=3):
    nc = bacc.Bacc("TRN2", target_bir_lowering=False, debug=False)

    sq01_d = {s: nc.dram_tensor(f"sq01_{s}", [BPC * DH, 2 * TPS], f16,
                                kind="ExternalInput")
              for s in "ab"}
    sq2_d = {s: nc.dram_tensor(f"sq2_{s}", [KC2, BPC * TPS], f16,
                               kind="ExternalInput")
             for s in "ab"}
    # additive mask, pair-batched layout: partition (side, smp%2, r),
    # free (pair, l) — the whole softmax for one PAIR of samples is a
    # single [4RV, RL] slice at partition 0
    mska_d = nc.dram_tensor("maskadd", [4 * RV, 2 * RL], f32,
                            kind="ExternalInput")
    w_d = nc.dram_tensor("wpack", [DH, 3 * DH], f16, kind="ExternalInput")

    out_v = {s: nc.dram_tensor(f"out_{s}", [RPC, DH], f32, kind="ExternalOutput")
             for s in "ab"}
    out_w = {s: nc.dram_tensor(f"outw_{s}", [RPC, RL], f32, kind="ExternalOutput")
             for s in "ab"}

    import contextlib
    outer_tc = tile.TileContext(nc) if not serial else None
    with (outer_tc if outer_tc is not None else contextlib.nullcontext()):
      for it_ in range(iters):
        pfx = f"i{it_}_" if iters > 1 else ""
        with (
            tile.TileContext(nc) if serial else contextlib.nullcontext()
        ) as maybe_tc:
          tc = maybe_tc if serial else outer_tc
          with (
            tc.For_i(0, loop_n, 1) if loop_n else contextlib.nullcontext()
          ):
           with (
            tc.tile_pool(name=pfx + "cst", bufs=1) as cst,
            tc.tile_pool(name=pfx + "seq", bufs=1) as seqp,
            tc.tile_pool(name=pfx + "ta", bufs=8) as tap,
            tc.tile_pool(name=pfx + "sm", bufs=1) as smp_pool,
            tc.tile_pool(name=pfx + "ps", bufs=2, space="PSUM") as ps,
        ):
            # ---- scalar (HWDGE) queue: lean consts, then the two c2
            # fat-row batches; per-sample tail smalls follow later.
            w_t = cst.tile([DH, 3 * DH], f16, tag="w", name=pfx + "wpack")
            nc.scalar.dma_start(w_t[:], w_d[:])
            mskf = cst.tile([4 * RV, 2 * RL], f32, tag="mska",
                            name=pfx + "mskf")
            nc.scalar.dma_start(mskf[:], mska_d[:])

            def w_lhs(c):
                kw = DH if c < 2 else KC2
                return w_t[0:kw, c * DH:(c + 1) * DH]

            # ---- bulk seq stream: per-sample [128, 2*TPS] tiles, side
            # b on sync, side a on gpsimd.  Each side's c2 batch rides
            # its own bulk queue right after sample 0 so FC(0) unblocks
            # early and the scalar queue stays small-only.
            sq01, sq2 = {}, {}
            qeng = {"b": nc.sync, "a": nc.gpsimd}

            def emit_sq01(s, smp):
                t01 = seqp.tile([DH, 2 * TPS], f16, tag="seq01",
                                bufs=4, name=f"{pfx}sq01_{s}{smp}")
                qeng[s].dma_start(
                    t01[:], sq01_d[s][smp * DH:(smp + 1) * DH, :])
                sq01[(s, smp)] = t01

            for s in "ab":
                emit_sq01(s, 0)
            for s in "ab":
                sq2[s] = seqp.tile([KC2, BPC * TPS], f16, tag="seq2",
                                   bufs=2, name=f"{pfx}sq2_{s}")
                qeng[s].dma_start(sq2[s][:], sq2_d[s][:])
            for smp in range(1, BPC):
                for s in "ab":
                    emit_sq01(s, smp)

            def sq_rhs(s, smp, c, n0, nw):
                if c < 2:
                    return sq01[(s, smp)][:, c * TPS + n0:c * TPS + n0 + nw]
                return sq2[s][:, smp * TPS + n0:smp * TPS + n0 + nw]

            taT, acc, mean16, aoutT = {}, {}, {}, {}
            for s in "ab":
                acc[s] = cst.tile([DH, BPC], f32, tag=f"acc{s}", name=f"{pfx}acc_{s}")
                mean16[s] = cst.tile([DH, BPC], f16, tag=f"mean{s}",
                                     name=f"{pfx}mean_{s}")
                aoutT[s] = cst.tile([DH, RPC], f32, tag=f"aoutT{s}",
                                    name=f"{pfx}aoutT_{s}")
            w2d40 = cst.tile([4 * RV, 2 * RL], f32, tag="w2d",
                             name=pfx + "w2d40")
            ones1 = cst.tile([1, DH], f32, tag="ones", name=pfx + "ones1")
            nc.vector.memset(ones1[:], 1.0)
            # identity for the epilogue PE transpose, built on-chip (no
            # 128-descriptor const DMA): ones everywhere, then keep only
            # the j == p diagonal via an affine iota select
            ones_sq = cst.tile([DH, DH], f32, tag="onsq", name=pfx + "ones_sq")
            nc.vector.memset(ones_sq[:], 1.0)
            ident_t = cst.tile([DH, DH], f32, tag="ident", name=pfx + "ident_t")
            nc.gpsimd.affine_select(
                ident_t[:], ones_sq[:], pattern=[[1, DH]],
                compare_op=ALU.is_equal, fill=0.0,
                base=0, channel_multiplier=-1)

            # ---- PE warm-up in two stages timed to end AT FC start:
            # stage 1 gates on the wpack DMA (~4us in), stage 2 on the
            # first seq tile (same dep as FC(0)), so the HAM clock gate
            # reaches 8/8 with no idle window before the real matmuls.
            wrhs = smp_pool.tile([DH, 512], f16, tag="wrhs", name=pfx + "wrhs")
            nc.vector.memset(wrhs[:], 0.0)
            if stage >= 1:
                wfc = ps.tile([DH, TPS], f32, tag="fc", bufs=2,
                              name=pfx + "warm_fc")
                for k in range(16):
                    nc.tensor.matmul(wfc[:, 0:512], w_lhs(0), wrhs[:],
                                     start=True, stop=True)
                for k in range(6):
                    nc.tensor.matmul(wfc[:, 0:512], w_lhs(0),
                                     sq01[("b", 0)][:, 0:512],
                                     start=True, stop=True)

            other = {"a": "b", "b": "a"}

            def emit_fc_pair(smp, inject=None):
                # inject: list of closures emitting PE-side tail work
                # (score matvec + copy pairs of the PREVIOUS sample),
                # slotted between c-groups so the PSUM-ring round trips
                # hide behind FC matmuls instead of stalling the PE.
                if stage < 1:
                    return
                pfc = {}
                for s in ("b", "a"):
                    pfc[s] = ps.tile([DH, TPS], f32, tag="fc", bufs=2,
                                     name=f"{pfx}pfc_{s}{smp}")
                    taT[(s, smp)] = tap.tile([DH, TPS], f16, tag="taT",
                                             name=f"{pfx}taT_{s}{smp}")
                # c-outer: 3 weight loads per sample pair instead of 18
                inj = list(inject or [])
                for c in range(3):
                    for s in ("b", "a"):
                        for n0, nw in NCH:
                            nc.tensor.matmul(
                                pfc[s][:, n0:n0 + nw],
                                w_lhs(c),
                                sq_rhs(s, smp, c, n0, nw),
                                start=(c == 0), stop=(c == 2))
                    for _ in range(2):
                        if inj:
                            inj.pop(0)()
                while inj:
                    inj.pop(0)()
                # relu + row-sum eviction: side b on ACT, side a on DVE
                # (halves the per-sample ACT time, which was the pacer)
                s = "b"
                nc.scalar.activation(
                    taT[(s, smp)][:], pfc[s][:], AF.Relu,
                    accum_out=acc[s][:, smp:smp + 1])
                nc.scalar.mul(mean16[s][:, smp:smp + 1],
                              acc[s][:, smp:smp + 1], 1.0 / TPS)
                s = "a"
                nc.vector.tensor_scalar_max(taT[(s, smp)][:], pfc[s][:], 0.0)
                nc.vector.reduce_sum(out=acc[s][:, smp:smp + 1],
                                     in_=taT[(s, smp)][:], axis=AX.X)
                nc.vector.tensor_scalar_mul(mean16[s][:, smp:smp + 1],
                                            acc[s][:, smp:smp + 1], 1.0 / TPS)

            srow_p = {}

            def score_ops(smp):
                # returns 6 closures, each one matvec chunk + its ACT
                # copy, to be interleaved between the NEXT sample's FC
                # c-groups
                if stage < 2:
                    return []
                p, j = smp // 2, smp % 2
                if j == 0:
                    srow_p[p] = smp_pool.tile([1, 4 * TPS], f32, tag="srow",
                                              bufs=2, name=f"{pfx}srow_p{p}")
                srow = srow_p[p]
                ops = []
                for i, s in enumerate(("a", "b")):
                    off = (i * 2 + j) * TPS
                    for ci, (n0, nw) in enumerate(NCH):
                        def op(i=i, s=s, ci=ci, n0=n0, nw=nw, off=off):
                            pscc = ps.tile([1, 512], f32, tag="wbc", bufs=2,
                                           name=f"{pfx}psc_{s}{smp}{ci}")
                            nc.tensor.matmul(
                                pscc[:, :nw],
                                mean16[other[s]][:, smp:smp + 1],
                                taT[(s, smp)][:, n0:n0 + nw])
                            nc.scalar.copy(
                                srow[:, off + n0:off + n0 + nw],
                                pscc[:, :nw])
                        ops.append(op)
                return ops

            def emit_soft_pair(p):
                # ONE batched masked softmax for both samples of the
                # pair: [4RV, RL] with partition = (side, smp%2, review).
                # Single chain of cross-engine hops per PAIR instead of
                # per sample.
                scs = smp_pool.tile([4 * RV, RL], f32, tag="scs", bufs=2,
                                    name=f"{pfx}scs_p{p}")
                nc.scalar.dma_start(scs[:], srow_p[p][:])
                lgs = smp_pool.tile([4 * RV, RL], f32, tag="lgs", bufs=2,
                                    name=f"{pfx}lgs_p{p}")
                nc.vector.tensor_tensor(
                    out=lgs[:], in0=scs[:],
                    in1=mskf[:, p * RL:(p + 1) * RL], op=ALU.add)
                negmax = smp_pool.tile([4 * RV, 1], f32, tag="negmax", bufs=2,
                                       name=f"{pfx}negmax_p{p}")
                nc.vector.reduce_max(out=negmax[:], in_=lgs[:],
                                     axis=AX.X, negate=True)
                e2d = smp_pool.tile([4 * RV, RL], f32, tag="e2d", bufs=2,
                                    name=f"{pfx}e2d_p{p}")
                ssum = smp_pool.tile([4 * RV, 1], f32, tag="ssum", bufs=2,
                                     name=f"{pfx}ssum_p{p}")
                nc.scalar.activation(e2d[:], lgs[:], AF.Exp, bias=negmax[:],
                                     accum_out=ssum[:])
                rec = smp_pool.tile([4 * RV, 1], f32, tag="rec", bufs=2,
                                    name=f"{pfx}rec_p{p}")
                nc.vector.reciprocal(rec[:], ssum[:])
                nc.vector.tensor_scalar_mul(
                    w2d40[:, p * RL:(p + 1) * RL], e2d[:], rec[:])
                # ship softmax weights (2 DMAs/pair; rows are already in
                # (smp, review)-major order per side) + the 4 flattened
                # weight rows for the weighted sums
                wrows = {}
                for i, s in enumerate(("a", "b")):
                    nc.scalar.dma_start(
                        out_w[s][p * 2 * RV:(p + 1) * 2 * RV, :],
                        w2d40[i * 2 * RV:(i + 1) * 2 * RV,
                              p * RL:(p + 1) * RL])
                    for j in range(2):
                        wr = smp_pool.tile([1, TPS], f32, tag=f"wrow{s}",
                                           bufs=2,
                                           name=f"{pfx}wrow_{s}{2 * p + j}")
                        nc.scalar.dma_start(
                            wr[:], w2d40[(i * 2 + j) * RV:
                                         (i * 2 + j + 1) * RV,
                                         p * RL:(p + 1) * RL])
                        wrows[(s, 2 * p + j)] = wr
                return wrows

            def emit_wsum(smp, wrow):
                if stage < 3:
                    return
                # weighted sums: PE ones-matmul broadcasts the weight row
                # into bank-sized PSUM chunks; DVE multiplies with
                # taT(fp16) into fp16 tmp; per-chunk segmented reduces
                # (chunks are review-aligned: 4+4+2 reviews)
                for i, s in enumerate(("a", "b")):
                    tmp = smp_pool.tile([DH, TPS], f16, tag="tmp", bufs=2,
                                        name=f"{pfx}tmp_{s}{smp}")
                    for ci, (n0, nw) in enumerate(NCH):
                        wbc = ps.tile([DH, 512], f32, tag="wbc", bufs=2,
                                      name=f"{pfx}wbc_{s}{smp}{ci}")
                        nc.tensor.matmul(
                            wbc[:, :nw],
                            ones1[:].bitcast(f32r),
                            wrow[s][:, n0:n0 + nw].bitcast(f32r))
                        nc.vector.tensor_tensor(
                            out=tmp[:, n0:n0 + nw],
                            in0=taT[(s, smp)][:, n0:n0 + nw],
                            in1=wbc[:, :nw], op=ALU.mult)
                        nc.vector.reduce_sum(
                            out=aoutT[s][:, smp * RV + n0 // RL:
                                         smp * RV + (n0 + nw) // RL],
                            in_=tmp[:, n0:n0 + nw].rearrange(
                                "p (r l) -> p r l", l=RL),
                            axis=AX.X)

            # Emission order: SC(s-1) interleaves into FC(s); pair 0's
            # batched softmax after FC(2), its weighted sums after
            # FC(3), so no PE instruction waits on anything later than
            # its own sample's data.
            if stage >= 2:
                emit_fc_pair(0)
                emit_fc_pair(1, inject=score_ops(0))
                emit_fc_pair(2, inject=score_ops(1))
                wr0 = emit_soft_pair(0)
                emit_fc_pair(3, inject=score_ops(2))
                for j in range(2):
                    emit_wsum(j, {s: wr0[(s, j)] for s in "ab"})
                for op in score_ops(3):
                    op()
                wr1 = emit_soft_pair(1)
                for j in range(2, 4):
                    emit_wsum(j, {s: wr1[(s, j)] for s in "ab"})
            else:
                for smp in range(BPC):
                    emit_fc_pair(smp)

            # ---- per-side epilogue: PE transpose (on-chip identity),
            # DVE copy out of PSUM, out_v on the idle sync queue
            for s in ("a", "b") if stage >= 3 else ():
                ptp = ps.tile([RPC, DH], f32, tag="wbc", bufs=2,
                              name=f"{pfx}ptp_{s}")
                nc.tensor.matmul(ptp[:], aoutT[s][:], ident_t[:],
                                 is_transpose=True)
                aout = smp_pool.tile([RPC, DH], f32, tag="aout",
                                     name=f"{pfx}aout_{s}")
                nc.vector.tensor_copy(aout[:], ptp[:])
                nc.sync.dma_start(out_v[s][:], aout[:])

    nc.compile()
    return nc


def build_in_maps(seq_a, seq_b, mask_a, mask_b, W, b):
    seq_a = np.asarray(seq_a, dtype=np.float32)
    seq_b = np.asarray(seq_b, dtype=np.float32)
    mask_a = np.asarray(mask_a, dtype=np.int32)
    mask_b = np.asarray(mask_b, dtype=np.int32)
    W = np.asarray(W, dtype=np.float32)
    b = np.asarray(b, dtype=np.float32)

    # W packed [128, 3*DH] fp16 with the bias folded in as c2 row 44
    wpack = np.zeros((DH, 3 * DH), dtype=np.float16)
    wpack[:, 0:DH] = W[0:DH]
    wpack[:, DH:2 * DH] = W[DH:2 * DH]
    wpack[0:DIN - 2 * DH, 2 * DH:3 * DH] = W[2 * DH:DIN]
    wpack[KC2 - 1, 2 * DH:3 * DH] = b

    in_maps = []
    for core in range(NCORES):
        b0 = core * BPC
        sl = {}
        for name, seq in (("a", seq_a), ("b", seq_b)):
            # [BPC, TPS, DIN] -> [BPC, DIN, TPS] fp16; c0|c1 of two
            # samples concatenated column-wise into [2*128, 4*TPS]; c2
            # batched across the 4 samples into [45, BPC*TPS] with a
            # trailing all-ones row (bias fold)
            chunk = (seq[b0:b0 + BPC].reshape(BPC, TPS, DIN)
                     .transpose(0, 2, 1).astype(np.float16))
            c01 = np.concatenate(
                [chunk[:, 0:DH, :], chunk[:, DH:2 * DH, :]], axis=2)
            sl[f"sq01_{name}"] = np.ascontiguousarray(
                c01.reshape(BPC * DH, 2 * TPS))
            c2 = chunk[:, 2 * DH:DIN, :].transpose(1, 0, 2).reshape(
                DIN - 2 * DH, BPC * TPS)
            sl[f"sq2_{name}"] = np.ascontiguousarray(np.concatenate(
                [c2, np.ones((1, BPC * TPS), dtype=np.float16)], axis=0))
        # [4RV, 2RL]: partition (side, smp%2, review), free (pair, token)
        msk = np.stack([m[b0:b0 + BPC].reshape(2, 2, RV, RL)
                        for m in (mask_a, mask_b)])  # [side, pair, j, r, l]
        msk = np.ascontiguousarray(
            msk.transpose(0, 2, 3, 1, 4).reshape(4 * RV, 2 * RL))
        sl["maskadd"] = np.where(msk > 0, 0.0, NEG_INF).astype(np.float32)
        sl["wpack"] = wpack
        in_maps.append(sl)
    return in_maps


def kernel(seq_a, seq_b, mask_a, mask_b, W, b):
    if "nc" not in _CACHE:
        _CACHE["nc"] = _build()
    nc = _CACHE["nc"]
    in_maps = build_in_maps(seq_a, seq_b, mask_a, mask_b, W, b)

    from concourse.bass_utils import run_bass_kernel_spmd
    res = run_bass_kernel_spmd(nc, in_maps, core_ids=list(range(NCORES)))
    _CACHE["last_result"] = res

    a_out = np.concatenate([r["out_a"] for r in res.results], axis=0)
    b_out = np.concatenate([r["out_b"] for r in res.results], axis=0)
    atob_w = np.concatenate([r["outw_a"] for r in res.results], axis=0)
    btoa_w = np.concatenate([r["outw_b"] for r in res.results], axis=0)
    return (a_out, b_out, atob_w, btoa_w)


# revision 37
# speedup vs baseline: 1.1909x; 1.0659x over previous
"""CoAttention kernel for Trainium2 (8 NeuronCores, data-parallel over batch).

Math (per sample): ta = relu(seq_a @ W + b), tb likewise.  The reference
mean-pools the [N, rv_len, M] affinity before softmax, and mean-pooling
commutes with the dot product:

    atob_scores[n, l] = mean_m( ta[n,l,:] . tb_all_tokens[m,:] )
                      = ta[n,l,:] . mean_m( tb_all_tokens[m,:] )

so each side only needs a dot with the *other side's per-sample mean
feature vector* — the 52M-element affinity tensor is never materialized.

v3 schedule.  Trace findings driving it: the DMA path is descriptor-rate
bound (~250-650ns per partition-row record per SDMA engine), so
per-partition rows must be fat and descriptor counts minimal; and the PE
queue is in-order, so a tail matmul whose input DMA is stuck behind bulk
traffic stalls all later FC matmuls.

- seq ships as fp16 (end-to-end rel-err ~4e-3 vs 2e-2 tolerance).
- sq01 (c0|c1) packs TWO samples per tile -> [128, 4*TPS] with 10KB
  rows; side b on sync, side a on gpsimd; 256 records per queue.
- c2 remainders ship per side as one [45, BPC*TPS] fat-row DMA on the
  scalar queue, where row 44 is all-ones: the FC bias is folded into
  the contraction (ta = [x|1] @ [W;b]), killing the [128,1]
  128-descriptor bias DMA.
- W ships as one [128, 3*DH] packed tile (128 records instead of 300);
  no identity matrix: the final transpose is a DVE 32x32 block
  transpose + block-permuted output DMAs.
- mask ships as an ADDITIVE f32 mask (0 / -1e9), so masking is one DVE
  add instead of memset+copy_predicated.
- scalar queue carries only: wpack, maskadd, sq2 x2, then per-sample
  tail smalls (scores reshape, out_w, weight rows) — a tail DMA never
  waits behind bulk.
- ~12 junk matmuls at t=0 warm the PE HAM clock gate (else the first
  ~3.4us of FC runs at 1.2GHz).
- weight broadcast is a PE ones-matmul into bank-sized PSUM chunks;
  DVE multiplies taT(fp16) into an fp16 tmp and does per-chunk
  segmented reduces into aoutT.
- PSUM: fc tag [128,1280] bufs=2 (6 banks, score matvec chunks ride
  the same ring), wbc tag [128,512] bufs=2 (2 banks) = exactly 8.
"""
import sys

sys.path.insert(0, "/opt/trn_rl_repo")

import numpy as np

import concourse.bacc as bacc
import concourse.tile as tile
from concourse import mybir

# Problem shape (hardcoded per contest contract)
BZ, RV, RL, DIN, DH = 32, 10, 128, 300, 128
NCORES = 8
BPC = BZ // NCORES            # samples per core: 4
TPS = RV * RL                 # tokens per sample: 1280
RPC = BPC * RV                # reviews per core: 40
NEG_INF = -1e9
KC2 = DIN - 2 * DH + 1        # c2 contraction rows incl. the ones row: 45

f32 = mybir.dt.float32
f32r = mybir.dt.float32r
f16 = mybir.dt.float16
AF = mybir.ActivationFunctionType
AX = mybir.AxisListType
ALU = mybir.AluOpType

# free-dim chunks of one sample's tokens (N <= 512 for one PSUM bank;
# chunk boundaries are review-aligned so segmented reduces stay clean)
NCH = [(0, 512), (512, 512), (1024, 256)]

_CACHE = {}


def _build(iters=1, serial=False, loop_n=0, stage=3):
    nc = bacc.Bacc("TRN2", target_bir_lowering=False, debug=False)

    sq01_d = {s: nc.dram_tensor(f"sq01_{s}", [BPC * DH, 2 * TPS], f16,
                                kind="ExternalInput")
              for s in "ab"}
    sq2_d = {s: nc.dram_tensor(f"sq2_{s}", [KC2, BPC * TPS], f16,
                               kind="ExternalInput")
             for s in "ab"}
    # additive mask, pair-batched layout: partition (side, smp%2, r),
    # free (pair, l) — the whole softmax for one PAIR of samples is a
    # single [4RV, RL] slice at partition 0
    mska_d = nc.dram_tensor("maskadd", [4 * RV, 2 * RL], f32,
                            kind="ExternalInput")
    w_d = nc.dram_tensor("wpack", [DH, 3 * DH], f16, kind="ExternalInput")

    out_v = {s: nc.dram_tensor(f"out_{s}", [RPC, DH], f32, kind="ExternalOutput")
             for s in "ab"}
    out_w = {s: nc.dram_tensor(f"outw_{s}", [RPC, RL], f32, kind="ExternalOutput")
             for s in "ab"}

    import contextlib
    outer_tc = tile.TileContext(nc) if not serial else None
    with (outer_tc if outer_tc is not None else contextlib.nullcontext()):
      for it_ in range(iters):
        pfx = f"i{it_}_" if iters > 1 else ""
        with (
            tile.TileContext(nc) if serial else contextlib.nullcontext()
        ) as maybe_tc:
          tc = maybe_tc if serial else outer_tc
          with (
            tc.For_i(0, loop_n, 1) if loop_n else contextlib.nullcontext()
          ):
           with (
            tc.tile_pool(name=pfx + "cst", bufs=1) as cst,
            tc.tile_pool(name=pfx + "seq", bufs=1) as seqp,
            tc.tile_pool(name=pfx + "ta", bufs=8) as tap,
            tc.tile_pool(name=pfx + "sm", bufs=1) as smp_pool,
            tc.tile_pool(name=pfx + "ps", bufs=2, space="PSUM") as ps,
        ):
            # ---- scalar (HWDGE) queue: lean consts, then the two c2
            # fat-row batches; per-sample tail smalls follow later.
            w_t = cst.tile([DH, 3 * DH], f16, tag="w", name=pfx + "wpack")
            nc.scalar.dma_start(w_t[:], w_d[:])
            mskf = cst.tile([4 * RV, 2 * RL], f32, tag="mska",
                            name=pfx + "mskf")
            nc.scalar.dma_start(mskf[:], mska_d[:])

            def w_lhs(c):
                kw = DH if c < 2 else KC2
                return w_t[0:kw, c * DH:(c + 1) * DH]

            # ---- bulk seq stream: per-sample [128, 2*TPS] tiles, side
            # b on sync, side a on gpsimd.  Each side's c2 batch rides
            # its own bulk queue right after sample 0 so FC(0) unblocks
            # early and the scalar queue stays small-only.
            sq01, sq2 = {}, {}
            qeng = {"b": nc.sync, "a": nc.gpsimd}

            def emit_sq01(s, smp):
                t01 = seqp.tile([DH, 2 * TPS], f16, tag="seq01",
                                bufs=4, name=f"{pfx}sq01_{s}{smp}")
                qeng[s].dma_start(
                    t01[:], sq01_d[s][smp * DH:(smp + 1) * DH, :])
                sq01[(s, smp)] = t01

            for s in "ab":
                emit_sq01(s, 0)
            for s in "ab":
                sq2[s] = seqp.tile([KC2, BPC * TPS], f16, tag="seq2",
                                   bufs=2, name=f"{pfx}sq2_{s}")
                qeng[s].dma_start(sq2[s][:], sq2_d[s][:])
            for smp in range(1, BPC):
                for s in "ab":
                    emit_sq01(s, smp)

            def sq_rhs(s, smp, c, n0, nw):
                if c < 2:
                    return sq01[(s, smp)][:, c * TPS + n0:c * TPS + n0 + nw]
                return sq2[s][:, smp * TPS + n0:smp * TPS + n0 + nw]

            taT, acc, mean16, aoutT = {}, {}, {}, {}
            for s in "ab":
                acc[s] = cst.tile([DH, BPC], f32, tag=f"acc{s}", name=f"{pfx}acc_{s}")
                mean16[s] = cst.tile([DH, BPC], f16, tag=f"mean{s}",
                                     name=f"{pfx}mean_{s}")
                aoutT[s] = cst.tile([DH, RPC], f32, tag=f"aoutT{s}",
                                    name=f"{pfx}aoutT_{s}")
            w2d40 = cst.tile([4 * RV, 2 * RL], f32, tag="w2d",
                             name=pfx + "w2d40")
            w2d16 = cst.tile([4 * RV, 2 * RL], f16, tag="w2d16",
                             name=pfx + "w2d16")
            ones1 = cst.tile([1, DH], f32, tag="ones", name=pfx + "ones1")
            nc.vector.memset(ones1[:], 1.0)
            # identity for the epilogue PE transpose, built on-chip (no
            # 128-descriptor const DMA): ones everywhere, then keep only
            # the j == p diagonal via an affine iota select
            ones_sq = cst.tile([DH, DH], f32, tag="onsq", name=pfx + "ones_sq")
            nc.vector.memset(ones_sq[:], 1.0)
            ident_t = cst.tile([DH, DH], f32, tag="ident", name=pfx + "ident_t")
            nc.gpsimd.affine_select(
                ident_t[:], ones_sq[:], pattern=[[1, DH]],
                compare_op=ALU.is_equal, fill=0.0,
                base=0, channel_multiplier=-1)

            # ---- PE warm-up in two stages timed to end AT FC start:
            # stage 1 gates on the wpack DMA (~4us in), stage 2 on the
            # first seq tile (same dep as FC(0)), so the HAM clock gate
            # reaches 8/8 with no idle window before the real matmuls.
            wrhs = smp_pool.tile([DH, 512], f16, tag="wrhs", name=pfx + "wrhs")
            nc.vector.memset(wrhs[:], 0.0)
            if stage >= 1:
                wfc = ps.tile([DH, TPS], f32, tag="fc", bufs=2,
                              name=pfx + "warm_fc")
                for k in range(16):
                    nc.tensor.matmul(wfc[:, 0:512], w_lhs(0), wrhs[:],
                                     start=True, stop=True)
                for k in range(6):
                    nc.tensor.matmul(wfc[:, 0:512], w_lhs(0),
                                     sq01[("b", 0)][:, 0:512],
                                     start=True, stop=True)

            other = {"a": "b", "b": "a"}

            def emit_fc_pair(smp, inject=None):
                # inject: list of closures emitting PE-side tail work
                # (score matvec + copy pairs of the PREVIOUS sample),
                # slotted between c-groups so the PSUM-ring round trips
                # hide behind FC matmuls instead of stalling the PE.
                if stage < 1:
                    return
                pfc = {}
                for s in ("b", "a"):
                    pfc[s] = ps.tile([DH, TPS], f32, tag="fc", bufs=2,
                                     name=f"{pfx}pfc_{s}{smp}")
                    taT[(s, smp)] = tap.tile([DH, TPS], f16, tag="taT",
                                             name=f"{pfx}taT_{s}{smp}")
                # c-outer: 3 weight loads per sample pair instead of 18
                inj = list(inject or [])
                for c in range(3):
                    for s in ("b", "a"):
                        for n0, nw in NCH:
                            nc.tensor.matmul(
                                pfc[s][:, n0:n0 + nw],
                                w_lhs(c),
                                sq_rhs(s, smp, c, n0, nw),
                                start=(c == 0), stop=(c == 2))
                    for _ in range(2):
                        if inj:
                            inj.pop(0)()
                while inj:
                    inj.pop(0)()
                # relu + row-sum eviction: side b on ACT, side a on DVE
                # (halves the per-sample ACT time, which was the pacer)
                s = "b"
                nc.scalar.activation(
                    taT[(s, smp)][:], pfc[s][:], AF.Relu,
                    accum_out=acc[s][:, smp:smp + 1])
                nc.scalar.mul(mean16[s][:, smp:smp + 1],
                              acc[s][:, smp:smp + 1], 1.0 / TPS)
                s = "a"
                nc.vector.tensor_scalar_max(taT[(s, smp)][:], pfc[s][:], 0.0)
                nc.vector.reduce_sum(out=acc[s][:, smp:smp + 1],
                                     in_=taT[(s, smp)][:], axis=AX.X)
                nc.vector.tensor_scalar_mul(mean16[s][:, smp:smp + 1],
                                            acc[s][:, smp:smp + 1], 1.0 / TPS)

            srow_p = {}

            def score_ops(smp):
                # returns 6 closures, each one matvec chunk + its ACT
                # copy, to be interleaved between the NEXT sample's FC
                # c-groups
                if stage < 2:
                    return []
                p, j = smp // 2, smp % 2
                if j == 0:
                    srow_p[p] = smp_pool.tile([1, 4 * TPS], f32, tag="srow",
                                              bufs=2, name=f"{pfx}srow_p{p}")
                srow = srow_p[p]
                ops = []
                for i, s in enumerate(("a", "b")):
                    off = (i * 2 + j) * TPS
                    for ci, (n0, nw) in enumerate(NCH):
                        def op(i=i, s=s, ci=ci, n0=n0, nw=nw, off=off):
                            pscc = ps.tile([1, 512], f32, tag="wbc", bufs=2,
                                           name=f"{pfx}psc_{s}{smp}{ci}")
                            nc.tensor.matmul(
                                pscc[:, :nw],
                                mean16[other[s]][:, smp:smp + 1],
                                taT[(s, smp)][:, n0:n0 + nw])
                            nc.scalar.copy(
                                srow[:, off + n0:off + n0 + nw],
                                pscc[:, :nw])
                        ops.append(op)
                return ops

            def emit_soft_pair(p):
                # ONE batched masked softmax for both samples of the
                # pair: [4RV, RL] with partition = (side, smp%2, review).
                # Single chain of cross-engine hops per PAIR instead of
                # per sample.
                scs = smp_pool.tile([4 * RV, RL], f32, tag="scs", bufs=2,
                                    name=f"{pfx}scs_p{p}")
                nc.scalar.dma_start(scs[:], srow_p[p][:])
                lgs = smp_pool.tile([4 * RV, RL], f32, tag="lgs", bufs=2,
                                    name=f"{pfx}lgs_p{p}")
                nc.vector.tensor_tensor(
                    out=lgs[:], in0=scs[:],
                    in1=mskf[:, p * RL:(p + 1) * RL], op=ALU.add)
                negmax = smp_pool.tile([4 * RV, 1], f32, tag="negmax", bufs=2,
                                       name=f"{pfx}negmax_p{p}")
                nc.vector.reduce_max(out=negmax[:], in_=lgs[:],
                                     axis=AX.X, negate=True)
                e2d = smp_pool.tile([4 * RV, RL], f32, tag="e2d", bufs=2,
                                    name=f"{pfx}e2d_p{p}")
                ssum = smp_pool.tile([4 * RV, 1], f32, tag="ssum", bufs=2,
                                     name=f"{pfx}ssum_p{p}")
                nc.scalar.activation(e2d[:], lgs[:], AF.Exp, bias=negmax[:],
                                     accum_out=ssum[:])
                rec = smp_pool.tile([4 * RV, 1], f32, tag="rec", bufs=2,
                                    name=f"{pfx}rec_p{p}")
                nc.vector.reciprocal(rec[:], ssum[:])
                nc.vector.tensor_scalar_mul(
                    w2d40[:, p * RL:(p + 1) * RL], e2d[:], rec[:])
                # fp16 copy of the weights feeds the gpsimd broadcast
                # path (HWDGE DMAs can't cast)
                nc.vector.tensor_scalar_mul(
                    w2d16[:, p * RL:(p + 1) * RL], e2d[:], rec[:])
                # ship softmax weights (2 DMAs/pair; rows are already in
                # (smp, review)-major order per side) + the 4 flattened
                # fp16 weight rows for the weighted sums
                wrows = {}
                for i, s in enumerate(("a", "b")):
                    nc.scalar.dma_start(
                        out_w[s][p * 2 * RV:(p + 1) * 2 * RV, :],
                        w2d40[i * 2 * RV:(i + 1) * 2 * RV,
                              p * RL:(p + 1) * RL])
                    for j in range(2):
                        wr = smp_pool.tile([1, TPS], f16, tag=f"wrow{s}",
                                           bufs=2,
                                           name=f"{pfx}wrow_{s}{2 * p + j}")
                        nc.scalar.dma_start(
                            wr[:], w2d16[(i * 2 + j) * RV:
                                         (i * 2 + j + 1) * RV,
                                         p * RL:(p + 1) * RL])
                        wrows[(s, 2 * p + j)] = wr
                return wrows

            def emit_wsum(smp, wrow):
                if stage < 3:
                    return
                # weighted sums: gpsimd (otherwise idle) broadcasts the
                # fp16 weight row across partitions; DVE multiplies with
                # taT(fp16) at 16-bit rate and does review-aligned
                # segmented reduces.  Pure forward pipeline — no PSUM
                # ring, no PE involvement.
                for i, s in enumerate(("a", "b")):
                    wbc = smp_pool.tile([DH, TPS], f16, tag="wbc16", bufs=2,
                                        name=f"{pfx}wbc_{s}{smp}")
                    nc.gpsimd.partition_broadcast(wbc[:], wrow[s][:])
                    tmp = smp_pool.tile([DH, TPS], f16, tag="tmp", bufs=2,
                                        name=f"{pfx}tmp_{s}{smp}")
                    nc.vector.tensor_tensor(
                        out=tmp[:], in0=taT[(s, smp)][:],
                        in1=wbc[:], op=ALU.mult)
                    for ci, (n0, nw) in enumerate(NCH):
                        nc.vector.reduce_sum(
                            out=aoutT[s][:, smp * RV + n0 // RL:
                                         smp * RV + (n0 + nw) // RL],
                            in_=tmp[:, n0:n0 + nw].rearrange(
                                "p (r l) -> p r l", l=RL),
                            axis=AX.X)

            # Emission order: SC(s-1) interleaves into FC(s); pair 0's
            # batched softmax after FC(2), its weighted sums after
            # FC(3), so no PE instruction waits on anything later than
            # its own sample's data.
            if stage >= 2:
                emit_fc_pair(0)
                emit_fc_pair(1, inject=score_ops(0))
                emit_fc_pair(2, inject=score_ops(1))
                wr0 = emit_soft_pair(0)
                emit_fc_pair(3, inject=score_ops(2))
                for j in range(2):
                    emit_wsum(j, {s: wr0[(s, j)] for s in "ab"})
                for op in score_ops(3):
                    op()
                wr1 = emit_soft_pair(1)
                for j in range(2, 4):
                    emit_wsum(j, {s: wr1[(s, j)] for s in "ab"})
            else:
                for smp in range(BPC):
                    emit_fc_pair(smp)

            # ---- per-side epilogue: PE transpose (on-chip identity),
            # DVE copy out of PSUM, out_v on the idle sync queue
            for s in ("a", "b") if stage >= 3 else ():
                ptp = ps.tile([RPC, DH], f32, tag="wbc", bufs=2,
                              name=f"{pfx}ptp_{s}")
                nc.tensor.matmul(ptp[:], aoutT[s][:], ident_t[:],
                                 is_transpose=True)
                aout = smp_pool.tile([RPC, DH], f32, tag="aout",
                                     name=f"{pfx}aout_{s}")
                nc.vector.tensor_copy(aout[:], ptp[:])
                nc.sync.dma_start(out_v[s][:], aout[:])

    nc.compile()
    return nc


def build_in_maps(seq_a, seq_b, mask_a, mask_b, W, b):
    seq_a = np.asarray(seq_a, dtype=np.float32)
    seq_b = np.asarray(seq_b, dtype=np.float32)
    mask_a = np.asarray(mask_a, dtype=np.int32)
    mask_b = np.asarray(mask_b, dtype=np.int32)
    W = np.asarray(W, dtype=np.float32)
    b = np.asarray(b, dtype=np.float32)

    # W packed [128, 3*DH] fp16 with the bias folded in as c2 row 44
    wpack = np.zeros((DH, 3 * DH), dtype=np.float16)
    wpack[:, 0:DH] = W[0:DH]
    wpack[:, DH:2 * DH] = W[DH:2 * DH]
    wpack[0:DIN - 2 * DH, 2 * DH:3 * DH] = W[2 * DH:DIN]
    wpack[KC2 - 1, 2 * DH:3 * DH] = b

    in_maps = []
    for core in range(NCORES):
        b0 = core * BPC
        sl = {}
        for name, seq in (("a", seq_a), ("b", seq_b)):
            # [BPC, TPS, DIN] -> [BPC, DIN, TPS] fp16; c0|c1 of two
            # samples concatenated column-wise into [2*128, 4*TPS]; c2
            # batched across the 4 samples into [45, BPC*TPS] with a
            # trailing all-ones row (bias fold)
            chunk = (seq[b0:b0 + BPC].reshape(BPC, TPS, DIN)
                     .transpose(0, 2, 1).astype(np.float16))
            c01 = np.concatenate(
                [chunk[:, 0:DH, :], chunk[:, DH:2 * DH, :]], axis=2)
            sl[f"sq01_{name}"] = np.ascontiguousarray(
                c01.reshape(BPC * DH, 2 * TPS))
            c2 = chunk[:, 2 * DH:DIN, :].transpose(1, 0, 2).reshape(
                DIN - 2 * DH, BPC * TPS)
            sl[f"sq2_{name}"] = np.ascontiguousarray(np.concatenate(
                [c2, np.ones((1, BPC * TPS), dtype=np.float16)], axis=0))
        # [4RV, 2RL]: partition (side, smp%2, review), free (pair, token)
        msk = np.stack([m[b0:b0 + BPC].reshape(2, 2, RV, RL)
                        for m in (mask_a, mask_b)])  # [side, pair, j, r, l]
        msk = np.ascontiguousarray(
            msk.transpose(0, 2, 3, 1, 4).reshape(4 * RV, 2 * RL))
        sl["maskadd"] = np.where(msk > 0, 0.0, NEG_INF).astype(np.float32)
        sl["wpack"] = wpack
        in_maps.append(sl)
    return in_maps


def kernel(seq_a, seq_b, mask_a, mask_b, W, b):
    if "nc" not in _CACHE:
        _CACHE["nc"] = _build()
    nc = _CACHE["nc"]
    in_maps = build_in_maps(seq_a, seq_b, mask_a, mask_b, W, b)

    from concourse.bass_utils import run_bass_kernel_spmd
    res = run_bass_kernel_spmd(nc, in_maps, core_ids=list(range(NCORES)))
    _CACHE["last_result"] = res

    a_out = np.concatenate([r["out_a"] for r in res.results], axis=0)
    b_out = np.concatenate([r["out_b"] for r in res.results], axis=0)
    atob_w = np.concatenate([r["outw_a"] for r in res.results], axis=0)
    btoa_w = np.concatenate([r["outw_b"] for r in res.results], axis=0)
    return (a_out, b_out, atob_w, btoa_w)
